# revision 40
# baseline (speedup 1.0000x reference)
"""Trainium2 Bass kernel for nn_ByteToLatentAttention.

Sharding: 8 cores = 2 (batch) x 4 (head-groups of 4 heads).  Each core
computes a partial output  attn_part @ wout_rows + merged_raw_rows @ wbyp_rows
for its batch; the host sums the 4 partials per batch and adds wout_b.
No device collectives needed.

Precision: the bypass path (raw x @ wbyp), which dominates the output
magnitude, runs in fp32 (float32r matmuls).  The attention pipeline
(norm'd x -> QKV -> rope -> softmax -> attn@V -> wout), whose contribution
to the output is ~0.5% of its magnitude, runs in bf16 with fp32 PSUM
accumulation.

Self-contained: hardcodes all shapes; uses only numpy + concourse.
"""

from contextlib import ExitStack

import numpy as np

import concourse.bass as bass
import concourse.tile as tile
from concourse import bacc
from concourse import mybir
from concourse.bass_utils import run_bass_kernel_spmd
from concourse.masks import make_identity

# ---- problem constants ----
B, S, D = 2, 4096, 512
BPL, H, DQK = 4, 16, 64
DLAT = 1024
LQ = S // BPL  # 1024
EPS = 1.1920929e-07
ROPE_BASE = 10000.0
NCORES = 8
NH = (H // 4) * DQK  # 256 features per core (4 heads)
P = 128

F32 = mybir.dt.float32
BF16 = mybir.dt.bfloat16
MM_F32 = mybir.dt.float32r  # full-rate PE path for 4-byte data

AF = mybir.ActivationFunctionType
ALU = mybir.AluOpType
AX = mybir.AxisListType


def _kernel_body(ctx: ExitStack, tc, io):
    nc = tc.nc

    const = ctx.enter_context(tc.tile_pool(name="const", bufs=1))
    work = ctx.enter_context(tc.tile_pool(name="work", bufs=2))
    evp = ctx.enter_context(tc.tile_pool(name="evp", bufs=4))
    stage = ctx.enter_context(tc.tile_pool(name="stage", bufs=1))
    psP = ctx.enter_context(tc.tile_pool(name="psP", bufs=2, space="PSUM"))
    psS = ctx.enter_context(tc.tile_pool(name="psS", bufs=2, space="PSUM"))
    psAcc = ctx.enter_context(tc.tile_pool(name="psAcc", bufs=1, space="PSUM"))
    psDen = ctx.enter_context(tc.tile_pool(name="psDen", bufs=1, space="PSUM"))

    ident = const.tile([P, P], F32)
    make_identity(nc, ident)
    ident_h = const.tile([P, P], BF16)
    nc.vector.tensor_copy(out=ident_h, in_=ident)
    ones64 = const.tile([P, 64], BF16)
    nc.vector.memset(ones64, 1.0)
    eps_sb = const.tile([P, 1], F32)
    nc.vector.memset(eps_sb, EPS)
    rot_sb = const.tile([P, P], BF16)
    nc.sync.dma_start(out=rot_sb, in_=io["rotm"])

    bq_sb = const.tile([P, 2], F32)
    nc.sync.dma_start(out=bq_sb, in_=io["bq"])
    bk_sb = const.tile([P, 2], F32)
    nc.sync.dma_start(out=bk_sb, in_=io["bk"])
    bv_sb = const.tile([P, 2], F32)
    nc.sync.dma_start(out=bv_sb, in_=io["bv"])

    # persistent big tensors
    normXT = const.tile([P, 4, S], BF16)  # [d_p, dc, s]   normalized x^T
    bypT = const.tile([P, 4, LQ], MM_F32)  # [d_p, dc, m]   raw bypass rows^T
    QTr = const.tile([P, 2, LQ], BF16)  # roped Q^T (pair hp -> heads 2hp,2hp+1)
    KTr = const.tile([P, 2, S], BF16)  # roped K^T
    acT = None  # allocated later from stage pool (tag q2)

    # ---------- phase 0: PE warm-up (HAM un-throttle) ----------
    warm_ps = psS.tile([P, 1024], F32, tag="sc")
    for wi in range(32):
        nc.tensor.matmul(
            warm_ps[:, 0:128], lhsT=ident_h, rhs=ident_h, start=True, stop=True
        )

    # ---------- phase 1: RMS norm in transposed domain ----------
    # xT (host-transposed, bf16): [d_p, dc, s]
    xT = stage.tile([P, 4, S], BF16, tag="A")
    nc.sync.dma_start(out=xT, in_=io["x_b"])
    # bypT (host-transposed raw rows, f32r): loaded directly
    nc.sync.dma_start(out=bypT, in_=io["x_byp"])
    ones128 = const.tile([P, P], BF16)
    nc.vector.memset(ones128, 1.0)
    for sc8 in range(8):
        ssl = slice(sc8 * 512, (sc8 + 1) * 512)
        pss = psP.tile([P, 512], F32, tag="mm")
        for dc in range(4):
            sq = work.tile([P, 512], BF16, tag="sq")
            if dc % 2 == 0:
                nc.scalar.square(out=sq, in_=xT[:, dc, ssl])
            else:
                nc.vector.tensor_mul(out=sq, in0=xT[:, dc, ssl], in1=xT[:, dc, ssl])
            nc.tensor.matmul(
                pss, lhsT=ones128, rhs=sq, start=(dc == 0), stop=(dc == 3)
            )
        rmsb = work.tile([P, 512], F32, tag="rmsb")
        nc.scalar.activation(
            out=rmsb, in_=pss, func=AF.Sqrt, bias=eps_sb, scale=1.0 / D
        )
        rinvf = work.tile([P, 512], F32, tag="rinvf")
        nc.vector.reciprocal_approx_fast(out=rinvf, in_=rmsb)
        for dc in range(4):
            nc.vector.tensor_mul(
                out=normXT[:, dc, ssl], in0=xT[:, dc, ssl], in1=rinvf
            )

    # ---------- phase 3: K projection + rope (bf16) ----------
    wk_sb = stage.tile([P, 4, NH], BF16, tag="w2")
    nc.sync.dma_start(out=wk_sb, in_=io["wk"])

    cs_k = stage.tile([P, 2, S], BF16, tag="B")
    nc.sync.dma_start(out=cs_k[:, 0, :], in_=io["cosk"])
    nc.sync.dma_start(out=cs_k[:, 1, :], in_=io["sink"])
    cosk_sb = cs_k[:, 0, :]
    sink_sb = cs_k[:, 1, :]
    for sf in range(8):
        ssl = slice(sf * 512, (sf + 1) * 512)
        ck = cosk_sb[:, ssl]
        sk = sink_sb[:, ssl]
        for mk in range(2):
            pk = psP.tile([P, 512], F32, tag="mm")
            for dc in range(4):
                nc.tensor.matmul(
                    pk,
                    lhsT=wk_sb[:, dc, mk * P : (mk + 1) * P],
                    rhs=normXT[:, dc, ssl],
                    start=(dc == 0),
                    stop=(dc == 3),
                )
            kb = work.tile([P, 512], BF16, tag="qb")
            nc.scalar.add(out=kb, in_=pk, add=bk_sb[:, mk : mk + 1])
            pr = psP.tile([P, 512], F32, tag="mm")
            nc.tensor.matmul(pr, lhsT=rot_sb, rhs=kb, start=True, stop=True)
            prh = work.tile([P, 512], BF16, tag="prh")
            nc.scalar.copy(out=prh, in_=pr)
            t1 = work.tile([P, 512], BF16, tag="t1")
            nc.vector.tensor_mul(out=t1, in0=kb, in1=ck)
            t2 = work.tile([P, 512], BF16, tag="t2")
            nc.vector.tensor_mul(out=t2, in0=prh, in1=sk)
            nc.vector.tensor_add(out=KTr[:, mk, ssl], in0=t1, in1=t2)

    # ---------- phase 2: Q projection + rope (bf16) ----------
    wq_sb = stage.tile([P, 16, NH], BF16, tag="W")
    nc.sync.dma_start(out=wq_sb, in_=io["wq"])
    cs_q = stage.tile([P, 2, LQ], BF16, tag="q2")
    nc.sync.dma_start(out=cs_q[:, 0, :], in_=io["cosq"])
    nc.sync.dma_start(out=cs_q[:, 1, :], in_=io["sinq"])
    cosq_sb = cs_q[:, 0, :]
    sinq_sb = cs_q[:, 1, :]

    for qf in range(2):
        for mq in range(2):
            qsl = slice(qf * 512, (qf + 1) * 512)
            pq = psP.tile([P, 512], F32, tag="mm")
            for sub in range(4):
                for dc in range(4):
                    kc = sub * 4 + dc
                    rhs = normXT[:, dc, :].rearrange("p (m s) -> p s m", s=4)[
                        :, sub, qsl
                    ]
                    nc.tensor.matmul(
                        pq,
                        lhsT=wq_sb[:, kc, mq * P : (mq + 1) * P],
                        rhs=rhs,
                        start=(kc == 0),
                        stop=(kc == 15),
                    )
            qb = work.tile([P, 512], BF16, tag="qb")
            nc.scalar.add(out=qb, in_=pq, add=bq_sb[:, mq : mq + 1])
            pr = psP.tile([P, 512], F32, tag="mm")
            nc.tensor.matmul(pr, lhsT=rot_sb, rhs=qb, start=True, stop=True)
            prh = work.tile([P, 512], BF16, tag="prh")
            nc.scalar.copy(out=prh, in_=pr)
            t1 = work.tile([P, 512], BF16, tag="t1")
            nc.vector.tensor_mul(out=t1, in0=qb, in1=cosq_sb[:, qsl])
            t2 = work.tile([P, 512], BF16, tag="t2")
            nc.vector.tensor_mul(out=t2, in0=prh, in1=sinq_sb[:, qsl])
            nc.vector.tensor_add(out=QTr[:, mq, qsl], in0=t1, in1=t2)

    # ---------- phase 4: V projection (bf16) ----------
    wv_sb = stage.tile([P, 4, NH], BF16, tag="w2")
    nc.sync.dma_start(out=wv_sb, in_=io["wv"])
    Vn = stage.tile([P, 32, NH], BF16, tag="A")  # [s_p, sc, n]  V natural
    for sc in range(S // P):
        pv = psP.tile([P, NH], F32, tag="mm")
        for dc in range(4):
            nc.tensor.matmul(
                pv,
                lhsT=normXT[:, dc, sc * P : (sc + 1) * P],
                rhs=wv_sb[:, dc, :],
                start=(dc == 0),
                stop=(dc == 3),
            )
        nc.scalar.copy(out=Vn[:, sc, :], in_=pv)

    # ---------- phase 5+6: attention, with out-proj interleaved per qc ----------
    acT = stage.tile([P, 2, LQ], BF16, tag="q2")  # attn output^T (+v bias)
    wo_sb = stage.tile([P, 2, DLAT], BF16, tag="w2")
    nc.sync.dma_start(out=wo_sb, in_=io["wo"])
    wb_sb = stage.tile([P, 4, DLAT], MM_F32, tag="W")
    nc.sync.dma_start(out=wb_sb, in_=io["wb"])

    def attention_qc(qc):
        qsl = slice(qc * 512, (qc + 1) * 512)
        for hp in range(2):
            pac = psAcc.tile([P, 512], F32, tag="pac")
            pden = psDen.tile([P, 512], F32, tag="pden")
            for sc in range(S // P):
                ksl = slice(sc * P, (sc + 1) * P)
                psab = psS.tile([P, 1024], F32, tag="sc")
                psa = psab[:, 0:512]
                psb = psab[:, 512:1024]
                nc.tensor.matmul(
                    psa,
                    lhsT=KTr[0:64, hp, ksl],
                    rhs=QTr[0:64, hp, qsl],
                    start=True,
                    stop=True,
                    skip_group_check=True,
                )
                nc.tensor.matmul(
                    psb,
                    lhsT=KTr[64:128, hp, ksl],
                    rhs=QTr[64:128, hp, qsl],
                    start=True,
                    stop=True,
                    skip_group_check=True,
                )
                eab = evp.tile([P, 1024], BF16, tag="ea", bufs=5)
                nc.scalar.activation(out=eab, in_=psab, func=AF.Exp, scale=0.125)
                ea = eab[:, 0:512]
                eb = eab[:, 512:1024]
                st, sp = (sc == 0), (sc == S // P - 1)
                cA = slice((2 * hp) * 64, (2 * hp) * 64 + 64)
                cB = slice((2 * hp + 1) * 64, (2 * hp + 1) * 64 + 64)
                nc.tensor.matmul(
                    pac[0:64, :], lhsT=Vn[:, sc, cA], rhs=ea,
                    start=st, stop=sp, tile_position=(0, 0), skip_group_check=True,
                )
                nc.tensor.matmul(
                    pac[64:128, :], lhsT=Vn[:, sc, cB], rhs=eb,
                    start=st, stop=sp, tile_position=(0, 64), skip_group_check=True,
                )
                if sc % 2 == 0:
                    eprev = eab
                else:
                    esum = work.tile([P, 1024], BF16, tag="esum")
                    nc.vector.tensor_add(out=esum, in0=eprev, in1=eab)
                    dst, dsp = (sc == 1), (sc == S // P - 1)
                    nc.tensor.matmul(
                        pden[0:64, :], lhsT=ones64, rhs=esum[:, 0:512],
                        start=dst, stop=dsp,
                        tile_position=(0, 0), skip_group_check=True,
                    )
                    nc.tensor.matmul(
                        pden[64:128, :], lhsT=ones64, rhs=esum[:, 512:1024],
                        start=dst, stop=dsp,
                        tile_position=(0, 64), skip_group_check=True,
                    )
            bc = work.tile([P, 512], F32, tag="bc")
            nc.vector.reciprocal_approx_fast(out=bc, in_=pden)
            tn = work.tile([P, 512], F32, tag="tn")
            nc.vector.tensor_mul(out=tn, in0=pac, in1=bc)
            nc.vector.tensor_scalar_add(
                out=acT[:, hp, qsl], in0=tn, scalar1=bv_sb[:, hp : hp + 1]
            )

    def outproj_qc(qc):
        for q8 in range(qc * 4, qc * 4 + 4):
            qsl8 = slice(q8 * P, (q8 + 1) * P)
            osb = evp.tile([P, DLAT], F32, tag="osb")
            for oc in range(2):
                osl = slice(oc * 512, (oc + 1) * 512)
                po = psP.tile([P, 512], F32, tag="mm")
                nc.tensor.matmul(
                    po, lhsT=acT[:, 0, qsl8], rhs=wo_sb[:, 0, osl],
                    start=True, stop=False,
                )
                nc.tensor.matmul(
                    po, lhsT=acT[:, 1, qsl8], rhs=wo_sb[:, 1, osl],
                    start=False, stop=False,
                )
                for dc in range(4):
                    nc.tensor.matmul(
                        po,
                        lhsT=bypT[:, dc, qsl8],
                        rhs=wb_sb[:, dc, osl],
                        start=False,
                        stop=(dc == 3),
                    )
                nc.vector.tensor_copy(out=osb[:, osl], in_=po)
            nc.sync.dma_start(out=io["out_partial"][qsl8, :], in_=osb)

    attention_qc(0)
    attention_qc(1)  # out-proj for qc=0 issued after: fills PE while qc=1 exps run
    outproj_qc(0)
    outproj_qc(1)

def build_program():
    nc = bacc.Bacc("TRN2", target_bir_lowering=False, debug=False)
    io = {}

    def inp(name, shape, dtype=F32):
        io[name] = nc.dram_tensor(name, list(shape), dtype, kind="ExternalInput").ap()

    inp("x_b", [P, 4, S], BF16)
    inp("x_byp", [P, 4, LQ], MM_F32)
    inp("wq", [P, 16, NH], BF16)
    inp("wk", [P, 4, NH], BF16)
    inp("wv", [P, 4, NH], BF16)
    inp("bq", [P, 2])
    inp("bk", [P, 2])
    inp("bv", [P, 2])
    inp("wo", [P, 2, DLAT], BF16)
    inp("wb", [P, 4, DLAT], MM_F32)
    inp("cosq", [P, LQ], BF16)
    inp("sinq", [P, LQ], BF16)
    inp("cosk", [P, S], BF16)
    inp("sink", [P, S], BF16)
    inp("rotm", [P, P], BF16)
    io["out_partial"] = nc.dram_tensor(
        "out_partial", [LQ, DLAT], F32, kind="ExternalOutput"
    ).ap()

    with tile.TileContext(nc) as tc:
        with ExitStack() as ctx:
            _kernel_body(ctx, tc, io)
    nc.compile()
    return nc


def _chunked_rows(w, dtype):
    """[C*128, N] -> [128, C, N] (partition-major chunks for direct DMA)."""
    c = w.shape[0] // P
    return np.ascontiguousarray(w.reshape(c, P, -1).transpose(1, 0, 2).astype(dtype))


def _rope_tables(pos):
    half = DQK // 2
    invfreq = ROPE_BASE ** (-np.arange(half, dtype=np.float64) / half)
    ang = pos[:, None].astype(np.float64) * invfreq[None, :]
    cos = np.cos(ang)
    sin = np.sin(ang)
    cos64 = np.concatenate([cos, cos], axis=1).T  # [64, L]
    sin64 = np.concatenate([-sin, sin], axis=1).T
    bf = np.dtype("bfloat16") if hasattr(np, "bfloat16") else None
    cosT = np.concatenate([cos64, cos64], axis=0)
    sinT = np.concatenate([sin64, sin64], axis=0)
    return cosT, sinT


def _tf32(a):
    u = np.ascontiguousarray(np.asarray(a, dtype=np.float32)).view(np.uint32)
    lsb = (u >> np.uint32(13)) & np.uint32(1)
    u = (u + np.uint32(0x0FFF) + lsb) & np.uint32(0xFFFFE000)
    return u.view(np.float32)


def _bf16(a):
    import ml_dtypes

    return np.ascontiguousarray(np.asarray(a).astype(ml_dtypes.bfloat16))


def make_in_map(core, inputs):
    b, hg = core // 4, core % 4
    x = np.asarray(inputs["x"], dtype=np.float32)
    nw = np.asarray(inputs["norm_w"], dtype=np.float32)
    wq_w = np.asarray(inputs["wq_w"], dtype=np.float32)
    wq_b = np.asarray(inputs["wq_b"], dtype=np.float32)
    wkv_w = np.asarray(inputs["wkv_w"], dtype=np.float32)
    wkv_b = np.asarray(inputs["wkv_b"], dtype=np.float32)
    wout_w = np.asarray(inputs["wout_w"], dtype=np.float32)
    wbyp_w = np.asarray(inputs["wbyp_w"], dtype=np.float32)

    import ml_dtypes

    BF = ml_dtypes.bfloat16
    nsl = slice(hg * NH, (hg + 1) * NH)
    vsl = slice(H * DQK + hg * NH, H * DQK + (hg + 1) * NH)
    wq_c = wq_w * np.tile(nw, BPL)[:, None]
    wkv_c = wkv_w * nw[:, None]

    cosq, sinq = _rope_tables(np.arange(LQ) * float(BPL))
    cosk, sink = _rope_tables(np.arange(S).astype(np.float64))

    rotm = np.zeros((P, P), dtype=np.float32)
    for m in range(P):
        blk, d = (m // 64) * 64, m % 64
        rotm[blk + (d + 32) % 64, m] = 1.0

    return {
        "x_b": _bf16(x[b].T.reshape(4, P, S).transpose(1, 0, 2)),
        "x_byp": _tf32(
            np.ascontiguousarray(x[b, hg::BPL, :].T.reshape(4, P, LQ).transpose(1, 0, 2))
        ),
        "wq": _chunked_rows(wq_c[:, nsl], BF),
        "wk": _chunked_rows(wkv_c[:, nsl], BF),
        "wv": _chunked_rows(wkv_c[:, vsl], BF),
        "bq": np.ascontiguousarray(wq_b[nsl].reshape(2, P).T),
        "bk": np.ascontiguousarray(wkv_b[nsl].reshape(2, P).T),
        "bv": np.ascontiguousarray(wkv_b[vsl].reshape(2, P).T),
        "wo": _chunked_rows(wout_w[nsl, :], BF),
        "wb": _tf32(_chunked_rows(wbyp_w[hg * D : (hg + 1) * D, :], np.float32)),
        "cosq": _bf16(cosq),
        "sinq": _bf16(sinq),
        "cosk": _bf16(cosk),
        "sink": _bf16(sink),
        "rotm": _bf16(rotm),
    }


_nc_cache = None


def _get_program():
    global _nc_cache
    if _nc_cache is None:
        _nc_cache = build_program()
    return _nc_cache


def run_device(inputs, trace=False):
    nc = _get_program()
    in_maps = [make_in_map(c, inputs) for c in range(NCORES)]
    res = run_bass_kernel_spmd(nc, in_maps, core_ids=list(range(NCORES)), trace=trace)
    return res


def assemble(parts, inputs):
    wout_b = np.asarray(inputs["wout_b"], dtype=np.float32)
    out = np.zeros((B, LQ, DLAT), dtype=np.float64)
    for c in range(NCORES):
        out[c // 4] += np.asarray(parts[c], dtype=np.float64)
    out += wout_b[None, None, :].astype(np.float64)
    return out.astype(np.float32)


def kernel(**inputs):
    res = run_device(inputs)
    parts = [r["out_partial"] for r in res.results]
    return assemble(parts, inputs)
